# revision 14
# baseline (speedup 1.0000x reference)
"""LIF spiking-neuron kernel for Trainium2, data-parallel over 8 NeuronCores.

Reference semantics (T=4, THRESH=1.0, TAU=1.0):
    x: [T*B, N] -> reshape [T, B, N]; mem0 = 0
    per t: mem += x_t; spike_t = (mem >= 1.0); mem *= (1 - spike_t)
    out: spikes reshaped [T*B, N]

Sharding: pure data parallel over B. Core i gets rows i*256:(i+1)*256 of
each timestep block -> shard x [T*256, N] = [1024, 4096] f32 per core.

The kernel is HBM/DMA-bound. Two levers vs the f32-store baseline
(~86 us, 33.5 MB/core of HBM traffic):
  1. Spikes are exactly {0,1}, so they are computed and stored as
     fp8-e4m3 (1.0 -> 0x38, lossless), cutting store traffic 4x;
     the host upcasts. Traffic: 16.78 MB reads + 4.19 MB writes.
  2. Double-width [128, 8192] tiles span both 128-row chunks of a
     timestep (free dim = (chunk, n)), halving DVE op count and DMA
     descriptor count: 4 loads of 4 MB + 4 stores of 1 MB per core.

Engine split (raw Bass):
  SP (sync) + ACT (scalar) HWDGE: x loads, alternating between the two
                                  DGE rings, NXB-deep tile ring
  gpsimd (SWDGE)                : fp8 spike stores
  DVE (vector), per t           :
      add   : mem += x_t            (skipped at t=0: mem0 + x0 = x0)
      spike : sb = (mem >= 1) ->fp8 (tensor_scalar is_ge)
      reset : mem = (mem < 1)*mem   (ONE fused scalar_tensor_tensor op,
                                     skipped at t=3 where mem is dead)

In lean mode (default) the DVE program carries no same-engine v_sem
self-waits — program order plus the hardware DRAIN already serialize DVE
RAW hazards — worth ~3 us/iter. Cross-engine waits (xb_sem load
completions, sb_sem store WAR) remain. Bit-packed spike outputs (pack /
pack_fp8 flags) were tried and are KEPT OFF: every extra DVE op costs
~6-8 us wall under concurrent DMA on this silicon, far above its idle
stream time, so the +4 pack ops cost more than the 3.7 MB store saving.
"""

from contextlib import ExitStack

import numpy as np

import concourse.bass as bass
from concourse import mybir
from concourse.bass_utils import run_bass_kernel_spmd

T = 4
B = 2048
N = 4096
N_CORES = 8
BSH = B // N_CORES  # 256 rows per core per timestep
P = 128

F32 = mybir.dt.float32
FP8 = mybir.dt.float8e4  # e4m3; 1.0 -> 0x38, 0.0 -> 0x00 (exact)
U8 = mybir.dt.uint8
PACK = False  # bit-pack spikes into one u8 plane [128, N] per core


def build_nc(t_dim=T, bsh=BSH, n=N, bench_iters=None, loads_2ring=True,
             nxb=4, nsb=3, mask_reset=False, pack=PACK, pack_fp8=False,
             lean=True):
    """One-core Bass module: x [t*bsh, n] f32 -> out [t*bsh, n] fp8.

    bench_iters: if set, repeat the whole (idempotent) program that many
    times with continuing semaphore counts — used only for slope timing.
    """
    pb = bsh // P  # 128-row chunks per timestep
    assert pb == 2 and t_dim == 4
    wn = pb * n  # 8192: both chunks side by side in the free dim
    reps = bench_iters or 1
    NXB = nxb  # x-tile ring (4 MB tiles)
    NSB = nsb  # spike-tile ring (1 MB fp8 tiles)

    nc = bass.Bass()
    x = nc.declare_dram_parameter("x", [t_dim * bsh, n], F32, isOutput=False)
    if pack_fp8:
        pack = False
        out = nc.declare_dram_parameter("out", [P, pb * n], FP8, isOutput=True)
        ov = None
    elif pack:
        out = nc.declare_dram_parameter("out", [P, n], U8, isOutput=True)
        ov = None
    else:
        out = nc.declare_dram_parameter(
            "out", [t_dim * bsh, n], FP8, isOutput=True
        )
        ov = out.rearrange("(t j p) n -> t p j n", t=t_dim, j=pb, p=P)
    xv = x.rearrange("(t j p) n -> t p j n", t=t_dim, j=pb, p=P)

    # DVE program order (v counts DVE instructions; each bumps v_sem by 1).
    # Per rep: t0: ge, reset | t1/t2: add, ge, reset | t3: add, ge.
    vidx_xfree = {}  # x-load j -> v count after the op that last reads it
    vidx_ge = {}  # store unit u -> v count after its is_ge
    vidx_merge = {}  # pack: rep g -> v count after its merge op
    v = 0
    for g in range(reps):
        for t in range(t_dim):
            j = t_dim * g + t
            if t == 0:
                v += 2
                vidx_xfree[j] = v
                vidx_ge[j] = v - 1
            else:
                v += 1  # add
                vidx_xfree[j] = v
                v += 1  # ge
                vidx_ge[j] = v
                if pack or pack_fp8:
                    v += 1  # pack STT
                if t < t_dim - 1:
                    v += 1  # reset
        if pack:
            v += 1  # merge halves
            vidx_merge[g] = v
        elif pack_fp8:
            vidx_merge[g] = v  # last pack of the rep

    with ExitStack() as ctx:
        mem = ctx.enter_context(nc.sbuf_tensor("mem", [P, wn], F32))
        xb = [
            ctx.enter_context(nc.sbuf_tensor(f"xb{i}", [P, wn], F32))
            for i in range(NXB)
        ]
        if pack_fp8:
            s8 = ctx.enter_context(nc.sbuf_tensor("s8", [P, wn], FP8))
            pkr = [
                ctx.enter_context(nc.sbuf_tensor(f"pkr{i}", [P, wn], FP8))
                for i in range(2)
            ]
            sb = None
        elif pack:
            s8 = ctx.enter_context(nc.sbuf_tensor("s8", [P, wn], U8))
            pk = ctx.enter_context(nc.sbuf_tensor("pk", [P, wn], U8))
            pkm = [
                ctx.enter_context(nc.sbuf_tensor(f"pkm{i}", [P, n], U8))
                for i in range(2)
            ]
            sb = None
        else:
            sb = [
                ctx.enter_context(nc.sbuf_tensor(f"sb{i}", [P, wn], FP8))
                for i in range(NSB)
            ]
        # One semaphore per ring slot: concurrent DMA completions interleave
        # their 16 per-engine increments, so a shared cumulative sem cannot
        # identify which DMA finished.
        xb_sem = [
            ctx.enter_context(nc.semaphore(f"xb_sem{i}")) for i in range(NXB)
        ]
        sb_sem = [
            ctx.enter_context(nc.semaphore(f"sb_sem{i}")) for i in range(NSB)
        ]
        v_sem = ctx.enter_context(nc.semaphore("v_sem"))
        block = ctx.enter_context(nc.Block())

        n_loads = t_dim * reps

        def load_body(engine, parity):
            for j in range(n_loads):
                if loads_2ring and j % 2 != parity:
                    continue
                t = j % t_dim
                if j >= NXB:  # WAR: x slot still read by DVE op
                    engine.wait_ge(v_sem, vidx_xfree[j - NXB])
                dst = xb[j % NXB][:].rearrange("p (j n) -> p j n", j=pb)
                engine.dma_start(dst, xv[t]).then_inc(xb_sem[j % NXB], 16)

        def store_body(engine):
            if pack_fp8:
                for g in range(reps):
                    engine.wait_ge(v_sem, vidx_merge[g])
                    engine.dma_start(out[:, :], pkr[g % 2][:]).then_inc(
                        sb_sem[g % 2], 16
                    )
                for i in range(2):
                    if reps > i:
                        engine.wait_ge(
                            sb_sem[i], 16 * ((reps - 1 - i) // 2 + 1)
                        )
                return
            if pack:
                for g in range(reps):
                    engine.wait_ge(v_sem, vidx_merge[g])
                    engine.dma_start(out[:, :], pkm[g % 2][:]).then_inc(
                        sb_sem[g % 2], 16
                    )
                for i in range(2):
                    if reps > i:
                        engine.wait_ge(
                            sb_sem[i], 16 * ((reps - 1 - i) // 2 + 1)
                        )
                return
            nu = t_dim * reps
            for u in range(nu):
                t = u % t_dim
                engine.wait_ge(v_sem, vidx_ge[u])
                ssrc = sb[u % NSB][:].rearrange("p (j n) -> p j n", j=pb)
                engine.dma_start(ov[t], ssrc).then_inc(sb_sem[u % NSB], 16)
            for i in range(NSB):  # drain: all stores landed before NEFF end
                if nu > i:
                    engine.wait_ge(sb_sem[i], 16 * ((nu - 1 - i) // NSB + 1))

        @block.sync
        def _(sync):
            load_body(sync, 0)

        if loads_2ring:

            @block.scalar
            def _(scalar):
                load_body(scalar, 1)

            @block.gpsimd
            def _(gp):
                store_body(gp)

        else:

            @block.scalar
            def _(scalar):
                store_body(scalar)

        @block.vector
        def _(vector):
            v = 0

            def dve(ins):
                nonlocal v
                v += 1
                ins.then_inc(v_sem, 1)

            def selfwait():
                # Redundant same-engine RAW guard (program order + DRAIN
                # already serialize DVE); kept only in non-lean mode.
                if not lean:
                    vector.wait_ge(v_sem, v)

            for g in range(reps):
                for t in range(t_dim):
                    j = t_dim * g + t
                    u = j
                    vector.wait_ge(xb_sem[j % NXB], 16 * (j // NXB + 1))
                    xt = xb[j % NXB]
                    src = xt if t == 0 else mem
                    if t > 0:
                        selfwait()
                        dve(vector.tensor_add(mem[:], mem[:], xt[:]))
                    if pack_fp8:
                        if t == 0:
                            # WAR: this rep's pk plane still being stored
                            if g >= 2:
                                vector.wait_ge(sb_sem[g % 2], 16 * (g // 2))
                            ge_dst = pkr[g % 2]
                        else:
                            ge_dst = s8
                    elif pack:
                        ge_dst = pk if t == 0 else s8
                    else:
                        if u >= NSB:  # WAR: spike slot still being stored
                            vector.wait_ge(sb_sem[u % NSB], 16 * (u // NSB))
                        ge_dst = sb[u % NSB]
                    selfwait()
                    dve(
                        vector.tensor_scalar(
                            ge_dst[:], src[:], 1.0, None,
                            mybir.AluOpType.is_ge,
                        )
                    )
                    if pack and t > 0:  # pk = pk*2 + s
                        selfwait()
                        dve(
                            vector.scalar_tensor_tensor(
                                pk[:], pk[:], 2.0, s8[:],
                                mybir.AluOpType.mult, mybir.AluOpType.add,
                            )
                        )
                    if pack_fp8 and t > 0:  # pk = pk*2 + s (fp8, <=15)
                        selfwait()
                        dve(
                            vector.scalar_tensor_tensor(
                                pkr[g % 2][:], pkr[g % 2][:], 2.0, s8[:],
                                mybir.AluOpType.mult, mybir.AluOpType.add,
                            )
                        )
                    if t < t_dim - 1:
                        selfwait()
                        if mask_reset:
                            # mem = (spike == 0) * mem; reads the fp8 spike
                            # tile instead of double-reading f32 src
                            dve(
                                vector.scalar_tensor_tensor(
                                    mem[:], sb[u % NSB][:], 0.0, src[:],
                                    mybir.AluOpType.is_equal,
                                    mybir.AluOpType.mult,
                                )
                            )
                        else:
                            dve(
                                vector.scalar_tensor_tensor(
                                    mem[:], src[:], 1.0, src[:],
                                    mybir.AluOpType.is_lt,
                                    mybir.AluOpType.mult,
                                )
                            )
                if pack:
                    # merge chunk halves: pkm = pk[:, :n]*16 + pk[:, n:]
                    if g >= 2:  # WAR: plane still being stored
                        vector.wait_ge(sb_sem[g % 2], 16 * (g // 2))
                    selfwait()
                    dve(
                        vector.scalar_tensor_tensor(
                            pkm[g % 2][:], pk[:, 0:n], 16.0, pk[:, n:wn],
                            mybir.AluOpType.mult, mybir.AluOpType.add,
                        )
                    )

    return nc


_NC_CACHE = None


def _get_nc():
    global _NC_CACHE
    if _NC_CACHE is None:
        _NC_CACHE = build_nc()
    return _NC_CACHE


def shard_input(x):
    """x [T*B, N] -> list of 8 shards [T*BSH, N], C-contiguous."""
    xs = x.reshape(T, B, N)
    return [
        np.ascontiguousarray(xs[:, i * BSH : (i + 1) * BSH, :]).reshape(T * BSH, N)
        for i in range(N_CORES)
    ]


def unshard_output(results):
    """8 per-core shards -> full f32 [T*B, N].

    Auto-detects layout by size: [T*BSH, N] fp8-e4m3 spikes (1.0 -> 0x38),
    or packed [P, N] u8 planes (bit (7-t) = spike[t, rows 0:128], bit
    (3-t) = spike[t, rows 128:256] of the core's 256-row slab).
    """
    out = np.empty((T, B, N), dtype=np.float32)
    for i in range(N_CORES):
        raw = np.asarray(results[i]).view(np.uint8)
        if raw.size == T * BSH * N:  # fp8 layout
            raw = raw.reshape(T, BSH, N)
            out[:, i * BSH : (i + 1) * BSH, :] = (raw == 0x38).astype(
                np.float32
            )
        elif raw.size == P * 2 * N:  # fp8 nibble plane [P, 2N]
            import ml_dtypes
            lutf = np.arange(256, dtype=np.uint8).view(
                ml_dtypes.float8_e4m3
            ).astype(np.float32)
            lut = np.where(
                (lutf >= 0) & (lutf <= 15) & (lutf == np.round(lutf)),
                lutf, 0
            ).astype(np.uint8)
            vals = lut[raw.reshape(P, 2 * N)]
            for ci in range(2):
                rows = slice(i * BSH + ci * P, i * BSH + (ci + 1) * P)
                nib = vals[:, ci * N : (ci + 1) * N]
                for t in range(T):
                    out[t, rows, :] = (nib >> (3 - t)) & 1
        else:  # packed u8 plane
            raw = raw.reshape(P, N)
            for ci in range(2):
                rows = slice(i * BSH + ci * P, i * BSH + (ci + 1) * P)
                for t in range(T):
                    shift = 4 * (1 - ci) + (3 - t)
                    out[t, rows, :] = (raw >> shift) & 1
    return out.reshape(T * B, N)


def run_sharded(x, trace=False):
    nc = _get_nc()
    in_maps = [{"x": s} for s in shard_input(x)]
    res = run_bass_kernel_spmd(nc, in_maps, list(range(N_CORES)), trace=trace)
    return unshard_output([r["out"] for r in res.results]), res


def kernel(x):
    x = np.asarray(x, dtype=np.float32)
    assert x.shape == (T * B, N)
    out, _ = run_sharded(x, trace=False)
    return out
